# revision 31
# baseline (speedup 1.0000x reference)
"""Trainium2 Bass kernel for a GPT-2 style transformer block (nn_Block_16690242913196).

Sharding strategy (8 NeuronCores, identical SPMD program):
  - QKV/proj/LN2/MLP: token-parallel. Core i owns 512 flat tokens.
  - Attention: head-parallel. Core i owns heads {2i, 2i+1} for ALL tokens.
  - Collective 1: AllGather of rstd-scaled activations + the mu*rstd row
    (bf16, ~1MB/rank). LN1 itself is folded into the QKV matmuls:
      ln1(x) @ Wg = (x*rstd) @ Wg - (mu*rstd) x c1 + 1 x cb
    with Wg = W * gamma (host-folded), c1 = colsum(Wg), cb = ln1_b @ W + b.
    The rank-1 corrections are two contract=1 matmuls appended to each
    PSUM accumulation chain, so no normalize pass exists on-chip and the
    AllGather fires ~15us into the kernel instead of ~70us.
  - Collective 2: AllToAll of attention outputs back to token-parallel.

  Activations stay feature-major ([D, tokens]) on-chip. Matmul operands are
  bf16; the residual spine and PSUM accumulation stay fp32. Softmax and LN
  reciprocals run on the Scalar engine (raw-emitted: the bass accuracy
  guard is irrelevant at this tolerance) instead of the iterative-divide
  DVE path. QKV for ranks 4-7 is emitted inside the batch-0 attention loop
  so the PE has matmul work under the exp (ACT) shadow.
"""

import numpy as np
import ml_dtypes

P = 128
B, S, D, H = 2, 2048, 1024, 16
DH = D // H          # 64
DI = 4 * D           # 4096
EPS = 1e-5
NCORES = 8
TT = B * S           # 4096 flat tokens
TOK = TT // NCORES   # 512 tokens per core
KD = D // P          # 8
KDI = DI // P        # 32
QCH = 256            # query chunk (2 blocks of 128)
NQC = S // QCH       # 8 query chunks per batch
HL = H // NCORES     # 2 local heads
RG = [list(range(NCORES))]
DGR = D + 2          # gathered rows: D of xs, mu*rstd row, ones row

_CACHED_NC = None


def build_nc():
    import concourse.bacc as bacc
    import concourse.tile as tile
    import concourse.mybir as mybir
    from contextlib import ExitStack

    dt = mybir.dt
    f32, bf16, f32r = dt.float32, dt.bfloat16, dt.float32r
    fp8 = dt.float8e4
    DR = mybir.MatmulPerfMode.DoubleRow
    AF = mybir.ActivationFunctionType
    OP = mybir.AluOpType

    nc = bacc.Bacc("TRN2", target_bir_lowering=False, debug=False,
                   num_devices=NCORES)

    # ---- kernel I/O (per-core shapes) ----
    xT = nc.dram_tensor("xT", [D, TOK], f32, kind="ExternalInput").ap()
    awg = nc.dram_tensor("awg", [P, KD, 3 * P], fp8, kind="ExternalInput").ap()
    cr = nc.dram_tensor("cr", [2, 3, P], fp8, kind="ExternalInput").ap()
    pw = nc.dram_tensor("pw", [KD, P, KD, P], bf16, kind="ExternalInput").ap()
    pb = nc.dram_tensor("pb", [P, KD], f32, kind="ExternalInput").ap()
    fw = nc.dram_tensor("fw", [KDI, P, KD, P], bf16, kind="ExternalInput").ap()
    fb = nc.dram_tensor("fb", [P, KDI], f32, kind="ExternalInput").ap()
    gw = nc.dram_tensor("gw", [KD, P, KDI, P], bf16, kind="ExternalInput").ap()
    gb = nc.dram_tensor("gb", [P, KD], f32, kind="ExternalInput").ap()
    l2g = nc.dram_tensor("l2g", [P, KD], f32, kind="ExternalInput").ap()
    l2b = nc.dram_tensor("l2b", [P, KD], f32, kind="ExternalInput").ap()
    mk = nc.dram_tensor("mk", [2, P, QCH], bf16, kind="ExternalInput").ap()
    sel = nc.dram_tensor("sel", [HL, P], bf16, kind="ExternalInput").ap()
    outT = nc.dram_tensor("outT", [D, TOK], f32, kind="ExternalOutput").ap()

    def act_raw(out, in_, func, bias=0.0, scale=1.0):
        """nc.scalar.activation minus the Reciprocal/Rsqrt accuracy guard.

        The ~2e-2 harness tolerance dwarfs the ScalarE approximation error,
        and moving these off the (iterative-divide) DVE path is worth it.
        """
        se = nc.scalar
        ins = [se.lower_ap(in_)]
        for arg in (bias, scale, 0.0):
            if isinstance(arg, float):
                ins.append(mybir.ImmediateValue(dtype=f32, value=arg))
            else:
                ins.append(se.lower_ap(arg))
        return se.add_instruction(mybir.InstActivation(
            name=se.bass.get_next_instruction_name(),
            func=func, ins=ins, outs=[se.lower_ap(out)]))

    with tile.TileContext(nc) as tc, ExitStack() as ctx:
        const = ctx.enter_context(tc.tile_pool(name="const", bufs=1))
        dram = ctx.enter_context(tc.tile_pool(name="dram", bufs=1, space="DRAM"))
        psum = ctx.enter_context(tc.tile_pool(name="psum", bufs=1, space="PSUM"))
        rows = ctx.enter_context(tc.tile_pool(name="rows", bufs=6))
        sqp = ctx.enter_context(tc.tile_pool(name="sqp", bufs=2))
        lnt = ctx.enter_context(tc.tile_pool(name="lnt", bufs=3))
        res = ctx.enter_context(tc.tile_pool(name="res", bufs=1))

        # ---- input + constants ----
        xT_sb = res.tile([P, KD, TOK], f32)
        xTv = xT.rearrange("(k p) c -> p k c", p=P)
        for k in range(KD):
            nc.sync.dma_start(xT_sb[:, k, :], xTv[:, k, :])

        awg_sb = const.tile([P, KD, 3 * P], fp8)
        nc.sync.dma_start(awg_sb, awg)
        cr_sb = const.tile([2, 3, P], fp8)
        nc.sync.dma_start(cr_sb, cr)
        mkc = const.tile([P, 2, QCH], bf16)
        nc.sync.dma_start(mkc, mk.rearrange("m p q -> p m q"))
        l2g_sb = const.tile([P, KD], f32)
        nc.sync.dma_start(l2g_sb, l2g)
        l2b_sb = const.tile([P, KD], f32)
        nc.sync.dma_start(l2b_sb, l2b)
        pb_sb = const.tile([P, KD], f32)
        nc.sync.dma_start(pb_sb, pb)
        fb_sb = const.tile([P, KDI], f32)
        nc.sync.dma_start(fb_sb, fb)
        gb_sb = const.tile([P, KD], f32)
        nc.sync.dma_start(gb_sb, gb)
        ones_cb = const.tile([P, 1], bf16)
        nc.vector.memset(ones_cb, 1.0)
        ones_tok_b = const.tile([1, TOK], bf16)
        nc.vector.memset(ones_tok_b, 1.0)
        ones_tok = const.tile([1, TOK], fp8)
        nc.vector.tensor_copy(ones_tok, ones_tok_b)
        ones_rf = const.tile([1, P], f32)
        nc.vector.memset(ones_rf, 1.0)
        ones_r = const.tile([1, P], f32r)
        nc.vector.tensor_copy(ones_r, ones_rf)
        ones_cf = const.tile([P, 1], f32)
        nc.vector.memset(ones_cf, 1.0)
        ones_c = const.tile([P, 1], f32r)
        nc.vector.tensor_copy(ones_c, ones_cf)
        eps_sb = const.tile([1, 1], f32)
        nc.vector.memset(eps_sb, EPS)
        sel2 = const.tile([HL, P], bf16)
        nc.sync.dma_start(sel2, sel)

        # collective bounce buffers
        cc1_in = dram.tile([DGR, TOK], fp8)
        cc1_out = dram.tile([NCORES, DGR, TOK], fp8, addr_space="Shared")
        cc2_in = dram.tile([NCORES, P, TOK], bf16)
        cc2_out = dram.tile([NCORES, P, TOK], bf16)
        cc2d_in = dram.tile([NCORES, HL, TOK], bf16)
        cc2d_out = dram.tile([NCORES, HL, TOK], bf16)

        # ---- phase 1: local LN1 stats, xs = x*rstd, mr = mu*rstd -> AG ----
        # sum(x) and sum(x^2) accumulate into two partition rows of ONE psum
        # bank; only the very first matmul carries start=True (a start
        # clears has_written for the whole bank).
        with nc.named_scope("ln1"):
            xb = res.tile([P, KD, TOK], bf16)
            sx1 = psum.tile([1, TOK], f32, tag="small", bufs=2)
            sx2 = psum.tile([1, TOK], f32, tag="small", bufs=2)
            for k in range(KD):
                nc.vector.tensor_copy(xb[:, k, :], xT_sb[:, k, :])
                sq = sqp.tile([P, TOK], bf16, tag="sq")
                nc.vector.tensor_mul(sq, xb[:, k, :], xb[:, k, :])
                nc.tensor.matmul(sx1, ones_cb, xb[:, k, :],
                                 start=(k == 0), stop=(k == KD - 1),
                                 skip_group_check=True)
                nc.tensor.matmul(sx2, ones_cb, sq,
                                 start=(k == 0), stop=(k == KD - 1),
                                 skip_group_check=True)
            mu = rows.tile([1, TOK], f32, tag="row")
            nc.vector.tensor_scalar_mul(mu, sx1, 1.0 / D)
            m2 = rows.tile([1, TOK], f32, tag="row")
            nc.vector.tensor_scalar_mul(m2, sx2, 1.0 / D)
            var = rows.tile([1, TOK], f32, tag="row")
            nc.vector.tensor_tensor(out=var, in0=mu, in1=mu, op=OP.mult)
            nc.vector.tensor_tensor(out=var, in0=m2, in1=var, op=OP.subtract)
            rstd = rows.tile([1, TOK], f32r, tag="row")
            act_raw(rstd, var, AF.Rsqrt, bias=eps_sb[:])
            mr = rows.tile([1, TOK], fp8, tag="mrow")
            nc.vector.tensor_tensor(out=mr, in0=mu, in1=rstd, op=OP.mult)
            nc.sync.dma_start(cc1_in[D:D + 1, :], mr)
            nc.sync.dma_start(cc1_in[D + 1:D + 2, :], ones_tok)
            rstd_b = psum.tile([P, TOK], f32, tag="big", bufs=2)
            nc.tensor.matmul(rstd_b, ones_r, rstd, start=True, stop=True)
            cc1v = cc1_in[0:D, :].rearrange("(k p) c -> p k c", p=P)
            xs8 = res.tile([P, KD, TOK], fp8)
            for k in range(KD):
                nc.vector.tensor_tensor(out=xs8[:, k, :], in0=xb[:, k, :],
                                        in1=rstd_b, op=OP.mult)
                nc.sync.dma_start(cc1v[:, k, :], xs8[:, k, :])
        with nc.named_scope("ag"):
            nc.gpsimd.collective_compute(
                "AllGather", OP.bypass, replica_groups=RG,
                ins=[cc1_in[:].opt()], outs=[cc1_out[:].opt()])

        # phase-scoped pools: attention-era tiles are freed before the MLP
        # pools (notably the 4MB hT) allocate.
        s1 = ExitStack()
        ares = s1.enter_context(tc.tile_pool(name="ares", bufs=1))
        xnp = s1.enter_context(tc.tile_pool(name="xnp", bufs=3))
        wp = s1.enter_context(tc.tile_pool(name="wp", bufs=6))
        psA = s1.enter_context(tc.tile_pool(name="psA", bufs=2, space="PSUM"))

        # ---- phase 2: QKV for local heads over all tokens ----
        kT = [ares.tile([P, TOK], bf16, name=f"kT{r}") for r in range(NCORES)]
        vA = [ares.tile([P, 4, HL, DH + 1], bf16, name=f"vA{r}")
              for r in range(NCORES)]
        qT = [ares.tile([P, TOK], bf16, name=f"qT{r}") for r in range(NCORES)]
        for r in range(NCORES):
            nc.vector.memset(vA[r][:, :, :, DH:DH + 1], 1.0)

        def emit_qkv(r):
            xs_r = xnp.tile([P, KD, TOK], fp8, tag="xnr")
            nc.sync.dma_start(xs_r, cc1_out[r, 0:D, :].rearrange(
                "(k p) c -> p k c", p=P))
            mro_r = xnp.tile([2, TOK], fp8, tag="mrr")
            nc.sync.dma_start(mro_r, cc1_out[r, D:DGR, :])
            for which in range(2):  # 0 -> q, 1 -> k
                ps = psum.tile([P, TOK], f32, tag="big", bufs=2)
                cb = which * P
                for kp in range(KD // 2):
                    nc.tensor.matmul(ps, awg_sb[:, 2 * kp:2 * kp + 2,
                                                cb:cb + P],
                                     xs_r[:, 2 * kp:2 * kp + 2, :],
                                     perf_mode=DR,
                                     start=(kp == 0), stop=False)
                nc.tensor.matmul(ps, cr_sb[:, which, :], mro_r,
                                 start=False, stop=True)
                dst = qT[r] if which == 0 else kT[r]
                nc.vector.tensor_scalar_mul(dst, ps, 1.0 / 16)
            for t in range(4):
                psv = psum.tile([P, P], f32, tag="big", bufs=2)
                for k in range(KD):
                    nc.tensor.matmul(psv, xs_r[:, k, P * t:P * (t + 1)],
                                     awg_sb[:, k, 2 * P:3 * P],
                                     start=(k == 0), stop=False)
                nc.tensor.matmul(psv, mro_r[:, P * t:P * (t + 1)],
                                 cr_sb[:, 2, :], start=False, stop=True)
                nc.vector.tensor_scalar_mul(
                    vA[r][:, t, :, 0:DH],
                    psv.rearrange("p (h d) -> p h d", h=HL), 1.0 / 16)

        # ---- phase 3: causal attention for local heads ----
        # Two k-blocks share one PSUM bank (halves ACT exp op count); the two
        # local heads run as independent chains so the PE stays dense.
        aT = ares.tile([P, NCORES, TOK], bf16)
        drows = ares.tile([1, NCORES, HL, TOK], bf16)

        def emit_attn(b, qc):
            qr = 4 * b + qc // 2
            qo = QCH * (qc % 2)
            nkb = 2 * qc + 2
            accs = [psA.tile([DH + 1, QCH], f32, tag="acc", bufs=2,
                             name=f"acc{b}_{qc}_{h}")
                    for h in range(HL)]
            for kb0 in range(0, nkb, 2):
                ws = []
                for h in range(HL):
                    hb = DH * h
                    sc = psA.tile([P, 2 * QCH], f32, tag="sc")
                    for j in range(2):
                        kb = kb0 + j
                        r = 4 * b + kb // 4
                        t = kb % 4
                        nc.tensor.matmul(
                            sc[:, QCH * j:QCH * (j + 1)],
                            kT[r][hb:hb + DH, P * t:P * (t + 1)],
                            qT[qr][hb:hb + DH, qo:qo + QCH],
                            start=True, stop=True,
                            skip_group_check=True)
                    w = wp.tile([P, 2 * QCH], bf16, tag="w")
                    nc.scalar.activation(w, sc, AF.Exp, scale=0.125)
                    if kb0 == 2 * qc:  # diagonal pair: apply masks
                        nc.vector.tensor_mul(
                            w.rearrange("p (m q) -> p m q", m=2),
                            w.rearrange("p (m q) -> p m q", m=2),
                            mkc)
                    ws.append(w)
                for h in range(HL):
                    for j in range(2):
                        kb = kb0 + j
                        r = 4 * b + kb // 4
                        t = kb % 4
                        nc.tensor.matmul(
                            accs[h], vA[r][:, t, h, :],
                            ws[h][:, QCH * j:QCH * (j + 1)],
                            start=(kb == 0), stop=(kb == nkb - 1),
                            skip_group_check=True)
            for h in range(HL):
                hb = DH * h
                acc = accs[h]
                nc.vector.tensor_copy(aT[hb:hb + DH, qr, qo:qo + QCH],
                                      acc[0:DH, :])
                nc.vector.tensor_copy(drows[0:1, qr, h, qo:qo + QCH],
                                      acc[DH:DH + 1, :])

        # Interleave: ranks 0-3 feed attention batch 0; ranks 4-7 are
        # emitted between batch-0 query chunks so the PE has matmul work
        # while the ACT engine chews through batch-0 exps.
        def ship(j):
            nc.sync.dma_start(cc2_in[j], aT[:, j, :])
            nc.sync.dma_start(
                cc2d_in[j].rearrange("h c -> (h c)"),
                drows[0:1, j, :, :].rearrange("o h c -> o (h c)"))

        with nc.named_scope("qkv"):
            for r in range(4):
                emit_qkv(r)
        with nc.named_scope("attn"):
            for qc in range(NQC):
                emit_attn(0, qc)
                if qc % 2 == 1:
                    ship(qc // 2)
            # Ranks 4-7 are emitted just-in-time inside the batch-1 loop:
            # rank 4+j is first consumed by attn(1, 2j), so emitting it
            # right before keeps batch-0's exp stream free of qkv matmuls
            # (the PE queue is strictly in-order) while still overlapping
            # the previous chunks' ACT work.
            with nc.named_scope("qkv2"):
                emit_qkv(4)
            for qc in range(NQC):
                emit_attn(1, qc)
                if qc % 2 == 1:
                    ship(4 + qc // 2)
                if qc in (0, 2, 4):
                    with nc.named_scope("qkv2"):
                        emit_qkv(5 + qc // 2)

        # ---- phase 4: AllToAll back to token-parallel ----
        with nc.named_scope("a2a"):
            nc.gpsimd.collective_compute(
                "AllToAll", OP.bypass, replica_groups=RG,
                ins=[cc2d_in[:].opt()], outs=[cc2d_out[:].opt()])
            nc.gpsimd.collective_compute(
                "AllToAll", OP.bypass, replica_groups=RG,
                ins=[cc2_in[:].opt()], outs=[cc2_out[:].opt()])

        s1.close()  # release attention-era SBUF
        psB = ctx.enter_context(tc.tile_pool(name="psB", bufs=2, space="PSUM"))
        mlp = ctx.enter_context(tc.tile_pool(name="mlp", bufs=1))
        wgt = ctx.enter_context(tc.tile_pool(name="wgt", bufs=1))
        outp = ctx.enter_context(tc.tile_pool(name="outp", bufs=2))

        # ---- phase 5: output projection + residual -> h1T (f32), with
        # LN2 stats accumulated chunk-by-chunk as proj produces them ----
        h1T = mlp.tile([P, KD, TOK], f32)
        st2b = psum.tile([1, TOK], f32, tag="small", bufs=2)
        st2c = psum.tile([1, TOK], f32, tag="small", bufs=2)
        with nc.named_scope("proj"):
            aF = [mlp.tile([P, TOK], bf16, name=f"aF{r}") for r in range(NCORES)]
            for r in range(NCORES):
                nc.sync.dma_start(aF[r], cc2_out[r])
            for r in range(NCORES):
                d2 = outp.tile([HL, TOK], bf16, tag="d2")
                nc.sync.dma_start(d2, cc2d_out[r])
                di = outp.tile([HL, TOK], bf16, tag="di")
                act_raw(di, d2, AF.Reciprocal)
                dib = psB.tile([P, TOK], f32, tag="bc", bufs=2)
                nc.tensor.matmul(dib, sel2, di, start=True, stop=True)
                nc.vector.tensor_tensor(out=aF[r], in0=aF[r], in1=dib,
                                        op=OP.mult)
            for f in range(KD):
                pwt = wgt.tile([P, KD, P], bf16, tag="pw", bufs=2)
                nc.sync.dma_start(pwt, pw[f])
                ps = psum.tile([P, TOK], f32, tag="big", bufs=2)
                for k in range(KD):
                    nc.tensor.matmul(ps, pwt[:, k, :], aF[k],
                                     start=(k == 0), stop=(k == KD - 1))
                t1 = lnt.tile([P, TOK], f32, tag="pj")
                nc.vector.tensor_scalar_add(t1, ps, pb_sb[:, f:f + 1])
                nc.vector.tensor_tensor(out=h1T[:, f, :], in0=t1,
                                        in1=xT_sb[:, f, :], op=OP.add)
                xr = sqp.tile([P, TOK], f32r, tag="xr")
                nc.vector.tensor_copy(xr, h1T[:, f, :])
                sq = sqp.tile([P, TOK], f32r, tag="sq2")
                nc.vector.tensor_mul(sq, h1T[:, f, :], h1T[:, f, :])
                nc.tensor.matmul(st2b, ones_c, xr,
                                 start=(f == 0), stop=(f == KD - 1),
                                 skip_group_check=True)
                nc.tensor.matmul(st2c, ones_c, sq,
                                 start=(f == 0), stop=(f == KD - 1),
                                 skip_group_check=True)

        # ---- phase 6: LN2 finalize -> mT (bf16) ----
        mT = mlp.tile([P, KD, TOK], bf16)
        with nc.named_scope("ln2"):
            mu2 = rows.tile([1, TOK], f32r, tag="row")
            nc.vector.tensor_scalar_mul(mu2, st2b, 1.0 / D)
            m22 = rows.tile([1, TOK], f32, tag="row")
            nc.vector.tensor_scalar_mul(m22, st2c, 1.0 / D)
            var2 = rows.tile([1, TOK], f32, tag="row")
            nc.vector.tensor_tensor(out=var2, in0=mu2, in1=mu2, op=OP.mult)
            nc.vector.tensor_tensor(out=var2, in0=m22, in1=var2,
                                    op=OP.subtract)
            rstd2 = rows.tile([1, TOK], f32r, tag="row")
            act_raw(rstd2, var2, AF.Rsqrt, bias=eps_sb[:])
            mub = psB.tile([P, TOK], f32, tag="bc", bufs=2)
            nc.tensor.matmul(mub, ones_r, mu2, start=True, stop=True)
            rsb = psB.tile([P, TOK], f32, tag="bc", bufs=2)
            nc.tensor.matmul(rsb, ones_r, rstd2, start=True, stop=True)
            for k in range(KD):
                t1 = lnt.tile([P, TOK], f32, tag="ln2")
                nc.vector.tensor_tensor(out=t1, in0=h1T[:, k, :], in1=mub,
                                        op=OP.subtract)
                nc.vector.tensor_tensor(out=t1, in0=t1, in1=rsb, op=OP.mult)
                nc.vector.tensor_scalar(out=mT[:, k, :], in0=t1,
                                        scalar1=l2g_sb[:, k:k + 1],
                                        scalar2=l2b_sb[:, k:k + 1],
                                        op0=OP.mult, op1=OP.add)

        # ---- phase 7: MLP ----
        hT = mlp.tile([P, KDI, TOK], bf16)
        with nc.named_scope("fc1"):
            for j in range(KDI):
                fwt = wgt.tile([P, KD, P], bf16, tag="fw", bufs=3)
                nc.sync.dma_start(fwt, fw[j])
                ps = psum.tile([P, TOK], f32, tag="big", bufs=2)
                for k in range(KD):
                    nc.tensor.matmul(ps, fwt[:, k, :], mT[:, k, :],
                                     start=(k == 0), stop=(k == KD - 1))
                nc.scalar.activation(hT[:, j, :], ps, AF.Gelu_apprx_tanh,
                                     bias=fb_sb[:, j:j + 1])
        with nc.named_scope("fc2"):
            for f in range(KD):
                gwt = wgt.tile([P, KDI, P], bf16, tag="gw", bufs=2)
                nc.sync.dma_start(gwt, gw[f])
                ps = psB.tile([P, TOK], f32, tag="f2", bufs=2)
                for k in range(KDI):
                    nc.tensor.matmul(ps, gwt[:, k, :], hT[:, k, :],
                                     start=(k == 0), stop=(k == KDI - 1))
                o = outp.tile([P, TOK], f32, tag="ot")
                nc.vector.tensor_scalar_add(o, ps, gb_sb[:, f:f + 1])
                nc.vector.tensor_tensor(out=o, in0=o, in1=h1T[:, f, :],
                                        op=OP.add)
                nc.sync.dma_start(outT[P * f:P * (f + 1), :], o)

    nc.compile()
    return nc


def shard_inputs(inputs):
    """Full inputs -> list of 8 per-core input dicts (host-side layout only)."""
    bf16 = ml_dtypes.bfloat16
    f32 = np.float32
    hs = np.asarray(inputs["hidden_states"], f32).reshape(TT, D)
    attn_w = np.asarray(inputs["attn_w"], f32)
    attn_b = np.asarray(inputs["attn_b"], f32)
    l1g = np.asarray(inputs["ln1_g"], f32)
    l1b = np.asarray(inputs["ln1_b"], f32)

    def col(v):  # [D] -> [P, KD]
        return np.ascontiguousarray(np.asarray(v, f32).reshape(KD, P).T)

    pw = np.ascontiguousarray(np.asarray(inputs["proj_w"], f32)
                              .reshape(KD, P, KD, P).transpose(2, 1, 0, 3)
                              .astype(bf16))
    f8 = ml_dtypes.float8_e4m3fn
    fw = np.ascontiguousarray(np.asarray(inputs["fc_w"], f32)
                              .reshape(KD, P, KDI, P).transpose(2, 1, 0, 3)
                              .astype(bf16))
    gw = np.ascontiguousarray(np.asarray(inputs["fc2_w"], f32)
                              .reshape(KDI, P, KD, P).transpose(2, 1, 0, 3)
                              .astype(bf16))
    pb = col(inputs["proj_b"])
    fbv = np.ascontiguousarray(np.asarray(inputs["fc_b"], f32)
                               .reshape(KDI, P).T)
    gbv = col(inputs["fc2_b"])
    l2g, l2b = col(inputs["ln2_g"]), col(inputs["ln2_b"])

    ii, jj = np.meshgrid(np.arange(P), np.arange(QCH), indexing="ij")
    mk = np.stack([(jj >= ii), (jj >= ii + P)]).astype(bf16)
    sel = np.zeros((HL, P), bf16)
    for h in range(HL):
        sel[h, DH * h:DH * (h + 1)] = 1

    maps = []
    for c in range(NCORES):
        cols = np.r_[P * c:P * (c + 1),
                     D + P * c:D + P * (c + 1),
                     2 * D + P * c:2 * D + P * (c + 1)]
        w_c = attn_w[:, cols]                      # [D, 384]
        wg_c = w_c * l1g[:, None]                  # gamma folded
        c1 = wg_c.sum(axis=0)                      # [384]
        cb = l1b @ w_c + attn_b[cols]              # [384]
        cr_c = np.ascontiguousarray((np.stack([
            np.stack([-c1[0:P], -c1[P:2 * P], -c1[2 * P:3 * P]]),
            np.stack([cb[0:P], cb[P:2 * P], cb[2 * P:3 * P]]),
        ]) * 16).astype(f8))
        awg_c = np.ascontiguousarray(
            (wg_c * 16).reshape(KD, P, 3 * P).transpose(1, 0, 2).astype(f8))
        xT_c = np.ascontiguousarray(hs[TOK * c:TOK * (c + 1)].T)
        maps.append({
            "xT": xT_c, "awg": awg_c, "cr": cr_c,
            "pw": pw, "pb": pb, "fw": fw, "fb": fbv, "gw": gw, "gb": gbv,
            "l2g": l2g, "l2b": l2b, "mk": mk, "sel": sel,
        })
    return maps


def unshard(results):
    out = np.concatenate([np.asarray(r["outT"]).T for r in results], axis=0)
    return np.ascontiguousarray(out.reshape(B, S, D))


def kernel(**inputs):
    global _CACHED_NC
    from concourse.bass_utils import run_bass_kernel_spmd
    if _CACHED_NC is None:
        _CACHED_NC = build_nc()
    in_maps = shard_inputs(inputs)
    res = run_bass_kernel_spmd(_CACHED_NC, in_maps,
                               core_ids=list(range(NCORES)))
    return unshard(res.results)
